# revision 12
# baseline (speedup 1.0000x reference)
"""Trainium2 Bass kernel for nn_NTfm3D: out[b,i,h,w] = sum_k masks[b,k,h,w] * (R[b,k] @ p[b,:,h,w] + t[b,k]).

Strategy (per core, data-parallel over batch B=16 across 8 cores, B_local=2):

Algebra:  out_i = sum_j S_ij * p~_j   where  S_ij = sum_k T[b,k,i,j] * m_k  (p~ = (x,y,z,1))
  mm1 (TensorE):  S = C_bd^T @ masks      (contract 8 masks per chunk, block-diag over 10 chunks)
  TT  (VectorE):  F = S * P3              (per-pixel multiply; P3 = points replicated x3 + ones rows)
  mm2 (TensorE):  out = O_bd^T @ F        (contract j, block-diag 0/1 weights)
  ACT (ScalarE):  psum -> sbuf staging;   DMA out.

Layout: channel-major tiles, N=512 pixel columns per matmul, C=10 chunks stacked on
partitions (S/P3/F use 120 partitions, masks 80). Host pre-packs all tiles into
contiguous per-group blocks (G groups of SUB sub-iters) so DMAs are large & contiguous.
"""

import numpy as np
from contextlib import ExitStack

import concourse.bass as bass
import concourse.tile as tile
from concourse import mybir
from concourse.bass_utils import run_bass_kernel_spmd

# ---- problem constants (hardcoded per contract) ----
B, K, H, W = 16, 8, 480, 640
HW = H * W                      # 307200
NCORES = 8
BL = B // NCORES                # 2 batches per core
N = 512                         # pixel columns per matmul
C = 10                          # chunks stacked per sub-iteration
SUB = 15                        # sub-iterations per group (one DMA batch)
G = HW // (N * C * SUB)         # 4 groups per batch plane
assert G * SUB * C * N == HW
FREE = SUB * N                  # 7680 free columns per group tile
RP = 9 * C                      # 90 product rows
PR = 12 * C                     # 120 partitions for S/P3/F
MR = 8 * C                      # 80 mask rows
OR = 3 * C                      # 30 output rows

# dtype config: "f16" (fast path) or "f32" (exact path)
DTYPE_CFG = "f16"

_CACHE = {}


def _dts(cfg):
    if cfg == "f16":
        return np.float16, mybir.dt.float16
    return np.float32, mybir.dt.float32


def _build_nc(cfg):
    np_dt, my_dt = _dts(cfg)
    f32 = mybir.dt.float32
    out_my = my_dt  # staging/output dtype matches input dtype config

    nc = bass.Bass()
    masks_d = nc.declare_dram_parameter("masks_p", [BL, G, MR, FREE], my_dt, isOutput=False)
    pts_d = nc.declare_dram_parameter("pts_p", [BL, G, PR, FREE], my_dt, isOutput=False)
    cbd_d = nc.declare_dram_parameter("cbd", [BL, MR, PR], my_dt, isOutput=False)
    obd_d = nc.declare_dram_parameter("obd", [PR, OR], my_dt, isOutput=False)
    out_d = nc.declare_dram_parameter("out_p", [BL, G, OR, FREE], out_my, isOutput=True)

    with ExitStack() as ctx:
        tc = ctx.enter_context(tile.TileContext(nc))
        wpool = ctx.enter_context(tc.tile_pool(name="wpool", bufs=1))
        cpool = ctx.enter_context(tc.tile_pool(name="cpool", bufs=2))
        mpool = ctx.enter_context(tc.tile_pool(name="mpool", bufs=2))
        ppool = ctx.enter_context(tc.tile_pool(name="ppool", bufs=2))
        fpool = ctx.enter_context(tc.tile_pool(name="fpool", bufs=4))
        ospool = ctx.enter_context(tc.tile_pool(name="ospool", bufs=2))
        s_psum = ctx.enter_context(tc.tile_pool(name="s_psum", bufs=3, space=bass.MemorySpace.PSUM))
        o_psum = ctx.enter_context(tc.tile_pool(name="o_psum", bufs=3, space=bass.MemorySpace.PSUM))

        obd_t = wpool.tile([PR, OR], my_dt)
        nc.gpsimd.dma_start(out=obd_t[:], in_=obd_d[:])

        for b in range(BL):
            cbd_t = cpool.tile([MR, PR], my_dt)
            nc.gpsimd.dma_start(out=cbd_t[:], in_=cbd_d[b])
            for g in range(G):
                mk = mpool.tile([MR, FREE], my_dt)
                nc.gpsimd.dma_start(out=mk[:], in_=masks_d[b, g])
                pt = ppool.tile([PR, FREE], my_dt)
                nc.gpsimd.dma_start(out=pt[:], in_=pts_d[b, g])
                ost = ospool.tile([OR, FREE], out_my)
                for it in range(SUB):
                    sl = slice(it * N, (it + 1) * N)
                    s_ps = s_psum.tile([PR, N], f32)
                    nc.tensor.matmul(s_ps[:], cbd_t[:], mk[:, sl], start=True, stop=True)
                    fp = fpool.tile([PR, N], my_dt)
                    nc.vector.tensor_mul(fp[:], s_ps[:], pt[:, sl])
                    o_ps = o_psum.tile([OR, N], f32)
                    nc.tensor.matmul(o_ps[:], obd_t[:], fp[:], start=True, stop=True)
                    nc.scalar.copy(ost[:, sl], o_ps[:])
                nc.gpsimd.dma_start(out=out_d[b, g], in_=ost[:])
    _split_excess_waits(nc)
    return nc


# Walrus's per-instruction sync-wait slots are limited (DVE tensor ops fit only ONE).
# Engine queues are strict FIFO, so hoisting extra waits onto injected same-engine
# NoOps immediately before the instruction is semantically identical.
_WAIT_LIMIT = {}
_WAIT_LIMIT_DEFAULT = 1
_SKIP_TYPES = {"InstEventSemaphore", "InstNoOp", "InstUnconditionalBranch"}


def _split_excess_waits(nc):
    uid = [0]
    for fn in nc.m.functions:
        for blk in fn.blocks:
            new_insts = []
            for inst in blk.instructions:
                nm = type(inst).__name__
                si = getattr(inst, "sync_info", None)
                eng = getattr(inst, "engine", None)
                if (
                    si is not None
                    and si.on_wait
                    and nm not in _SKIP_TYPES
                    and eng is not None
                ):
                    limit = _WAIT_LIMIT.get(nm, _WAIT_LIMIT_DEFAULT)
                    waits = list(si.on_wait)
                    if len(waits) > limit:
                        keep = waits[-limit:]
                        extra = waits[:-limit]
                        for w in extra:
                            uid[0] += 1
                            nop = mybir.InstNoOp(
                                name=f"wsplit-{uid[0]}",
                                engine=eng,
                                sync_info=mybir.SyncInfo(on_wait=[w], on_update=[]),
                                bass_nofuse=True,
                            )
                            new_insts.append(nop)
                        inst.sync_info = mybir.SyncInfo(on_wait=keep, on_update=list(si.on_update))
                new_insts.append(inst)
            blk.instructions = new_insts


def get_nc(cfg=DTYPE_CFG):
    if cfg not in _CACHE:
        _CACHE[cfg] = _build_nc(cfg)
    return _CACHE[cfg]


# ---------- host-side packing ----------

def _pack_masks(m, np_dt):
    # m: (BL, K, H, W) float32 -> (BL, G, MR=8q+k, FREE=it*N+n)
    a = m.reshape(BL, K, G, SUB, C, N).astype(np_dt)        # b k g it q n
    a = a.transpose(0, 2, 4, 1, 3, 5)                       # b g q k it n
    return np.ascontiguousarray(a.reshape(BL, G, MR, FREE))


def _pack_points(p, np_dt):
    # p: (BL, 3, H, W) -> (BL, G, PR, FREE); rows 9q+3j+r = p_j (r replicated), rows 90+ = 1.0
    a = p.reshape(BL, 3, G, SUB, C, N).astype(np_dt)        # b j g it q n
    a = a.transpose(0, 2, 4, 1, 3, 5)                       # b g q j it n
    a = np.broadcast_to(a[:, :, :, :, None, :, :], (BL, G, C, 3, 3, SUB, N))  # b g q j r it n
    a = a.reshape(BL, G, RP, FREE)
    ones = np.ones((BL, G, PR - RP, FREE), dtype=np_dt)
    return np.ascontiguousarray(np.concatenate([a, ones], axis=2))


def _make_cbd(t, np_dt):
    # t: (BL, K, 3, 4) transforms -> (BL, MR, PR)
    cbd = np.zeros((BL, MR, PR), dtype=np.float32)
    rot = t[:, :, :, :3].transpose(0, 1, 3, 2).reshape(BL, K, 9)   # b k (j r): value T[b,k,r,j]
    tr = t[:, :, :, 3]                                             # b k i
    for q in range(C):
        cbd[:, 8 * q:8 * q + 8, 9 * q:9 * q + 9] = rot
        cbd[:, 8 * q:8 * q + 8, RP + 3 * q:RP + 3 * q + 3] = tr
    return cbd.astype(np_dt)


def _make_obd(np_dt):
    obd = np.zeros((PR, OR), dtype=np.float32)
    for q in range(C):
        for i in range(3):
            for j in range(3):
                obd[9 * q + 3 * j + i, 3 * q + i] = 1.0
            obd[RP + 3 * q + i, 3 * q + i] = 1.0
    return obd.astype(np_dt)


def _unpack_out(o):
    # o: (BL, G, OR=3q+i, FREE=it*N+n) -> (BL, 3, H, W) float32
    a = np.asarray(o).reshape(BL, G, C, 3, SUB, N)          # b g q i it n
    a = a.transpose(0, 3, 1, 4, 2, 5)                       # b i g it q n
    return a.reshape(BL, 3, H, W).astype(np.float32)


def make_in_maps(points, masks, transforms, cfg=DTYPE_CFG):
    np_dt, _ = _dts(cfg)
    points = np.asarray(points, dtype=np.float32)
    masks = np.asarray(masks, dtype=np.float32)
    transforms = np.asarray(transforms, dtype=np.float32)
    obd = _make_obd(np_dt)
    in_maps = []
    for core in range(NCORES):
        b0 = core * BL
        in_maps.append({
            "masks_p": _pack_masks(masks[b0:b0 + BL], np_dt),
            "pts_p": _pack_points(points[b0:b0 + BL], np_dt),
            "cbd": _make_cbd(transforms[b0:b0 + BL], np_dt),
            "obd": obd,
        })
    return in_maps


def kernel(points, masks, transforms, cfg=DTYPE_CFG, **run_kwargs):
    nc = get_nc(cfg)
    in_maps = make_in_maps(points, masks, transforms, cfg)
    res = run_bass_kernel_spmd(nc, in_maps, list(range(NCORES)), **run_kwargs)
    outs = [_unpack_out(res.results[c]["out_p"]) for c in range(NCORES)]
    full = np.concatenate(outs, axis=0)
    if hasattr(kernel, "_last"):
        pass
    kernel._last_results = res
    return full


# revision 13
# speedup vs baseline: 2.0761x; 2.0761x over previous
"""Trainium2 Bass kernel for nn_NTfm3D: out[b,i,h,w] = sum_k masks[b,k,h,w] * (R[b,k] @ p[b,:,h,w] + t[b,k]).

Strategy (per core, data-parallel over batch B=16 across 8 cores, B_local=2):

Algebra:  out_i = sum_j S_ij * p~_j   where  S_ij = sum_k T[b,k,i,j] * m_k  (p~ = (x,y,z,1))
  mm1 (TensorE):  S = C_bd^T @ masks      (contract 8 masks per chunk, block-diag over 10 chunks)
  TT  (VectorE):  F = S * P3              (per-pixel multiply; P3 = points replicated x3 + ones rows)
  mm2 (TensorE):  out = O_bd^T @ F        (contract j, block-diag 0/1 weights)
  ACT (ScalarE):  psum -> sbuf staging;   DMA out.

Layout: channel-major tiles, N=512 pixel columns per matmul, C=10 chunks stacked on
partitions (S/P3/F use 120 partitions, masks 80). Host pre-packs all tiles into
contiguous per-group blocks (G groups of SUB sub-iters) so DMAs are large & contiguous.
"""

import numpy as np
from contextlib import ExitStack

import concourse.bass as bass
import concourse.tile as tile
from concourse import mybir
from concourse.bass_utils import run_bass_kernel_spmd

# ---- problem constants (hardcoded per contract) ----
B, K, H, W = 16, 8, 480, 640
HW = H * W                      # 307200
NCORES = 8
BL = B // NCORES                # 2 batches per core
N = 512                         # pixel columns per matmul
C = 10                          # chunks stacked per sub-iteration
SUB = 15                        # sub-iterations per group (one DMA batch)
G = HW // (N * C * SUB)         # 4 groups per batch plane
assert G * SUB * C * N == HW
FREE = SUB * N                  # 7680 free columns per group tile
RP = 9 * C                      # 90 product rows
PR = 12 * C                     # 120 partitions for S/P3/F
MR = 8 * C                      # 80 mask rows
OR = 3 * C                      # 30 output rows

# dtype config: "f16" (fast path) or "f32" (exact path)
DTYPE_CFG = "f16"

_CACHE = {}


def _dts(cfg):
    if cfg == "f16":
        return np.float16, mybir.dt.float16
    return np.float32, mybir.dt.float32


def _build_nc(cfg):
    np_dt, my_dt = _dts(cfg)
    f32 = mybir.dt.float32
    out_my = my_dt  # staging/output dtype matches input dtype config

    nc = bass.Bass()
    masks_d = nc.declare_dram_parameter("masks_p", [BL, G, MR, FREE], my_dt, isOutput=False)
    pts_d = nc.declare_dram_parameter("pts_p", [BL, G, PR, FREE], my_dt, isOutput=False)
    cbd_d = nc.declare_dram_parameter("cbd", [BL, MR, PR], my_dt, isOutput=False)
    obd_d = nc.declare_dram_parameter("obd", [PR, OR], my_dt, isOutput=False)
    out_d = nc.declare_dram_parameter("out_p", [BL, G, OR, FREE], out_my, isOutput=True)

    with ExitStack() as ctx:
        tc = ctx.enter_context(tile.TileContext(nc))
        wpool = ctx.enter_context(tc.tile_pool(name="wpool", bufs=1))
        cpool = ctx.enter_context(tc.tile_pool(name="cpool", bufs=2))
        mpool = ctx.enter_context(tc.tile_pool(name="mpool", bufs=2))
        ppool = ctx.enter_context(tc.tile_pool(name="ppool", bufs=2))
        fpool = ctx.enter_context(tc.tile_pool(name="fpool", bufs=4))
        ospool = ctx.enter_context(tc.tile_pool(name="ospool", bufs=2))
        s_psum = ctx.enter_context(tc.tile_pool(name="s_psum", bufs=3, space=bass.MemorySpace.PSUM))
        o_psum = ctx.enter_context(tc.tile_pool(name="o_psum", bufs=3, space=bass.MemorySpace.PSUM))

        obd_t = wpool.tile([PR, OR], my_dt)
        nc.gpsimd.dma_start(out=obd_t[:], in_=obd_d[:])

        for b in range(BL):
            cbd_t = cpool.tile([MR, PR], my_dt)
            nc.gpsimd.dma_start(out=cbd_t[:], in_=cbd_d[b])
            for g in range(G):
                mk = mpool.tile([MR, FREE], my_dt)
                nc.gpsimd.dma_start(out=mk[:], in_=masks_d[b, g])
                pt = ppool.tile([PR, FREE], my_dt)
                nc.gpsimd.dma_start(out=pt[:], in_=pts_d[b, g])
                ost = ospool.tile([OR, FREE], out_my)
                for it in range(SUB):
                    sl = slice(it * N, (it + 1) * N)
                    s_ps = s_psum.tile([PR, N], f32)
                    nc.tensor.matmul(s_ps[:], cbd_t[:], mk[:, sl], start=True, stop=True)
                    fp = fpool.tile([PR, N], my_dt)
                    nc.vector.tensor_mul(fp[:], s_ps[:], pt[:, sl])
                    o_ps = o_psum.tile([OR, N], f32)
                    nc.tensor.matmul(o_ps[:], obd_t[:], fp[:], start=True, stop=True)
                    nc.scalar.copy(ost[:, sl], o_ps[:])
                nc.gpsimd.dma_start(out=out_d[b, g], in_=ost[:])
    _split_excess_waits(nc)
    return nc


# Walrus's per-instruction sync-wait slots are limited (DVE tensor ops fit only ONE).
# Engine queues are strict FIFO, so hoisting extra waits onto injected same-engine
# NoOps immediately before the instruction is semantically identical.
_WAIT_LIMIT = {}
_WAIT_LIMIT_DEFAULT = 1
_SKIP_TYPES = {"InstEventSemaphore", "InstNoOp", "InstUnconditionalBranch"}


def _split_excess_waits(nc):
    uid = [0]
    for fn in nc.m.functions:
        for blk in fn.blocks:
            new_insts = []
            for inst in blk.instructions:
                nm = type(inst).__name__
                si = getattr(inst, "sync_info", None)
                eng = getattr(inst, "engine", None)
                if (
                    si is not None
                    and si.on_wait
                    and nm not in _SKIP_TYPES
                    and eng is not None
                ):
                    limit = _WAIT_LIMIT.get(nm, _WAIT_LIMIT_DEFAULT)
                    waits = list(si.on_wait)
                    if len(waits) > limit:
                        keep = waits[-limit:]
                        extra = waits[:-limit]
                        for w in extra:
                            uid[0] += 1
                            nop = mybir.InstNoOp(
                                name=f"wsplit-{uid[0]}",
                                engine=eng,
                                sync_info=mybir.SyncInfo(on_wait=[w], on_update=[]),
                                bass_nofuse=True,
                            )
                            new_insts.append(nop)
                        inst.sync_info = mybir.SyncInfo(on_wait=keep, on_update=list(si.on_update))
                new_insts.append(inst)
            blk.instructions = new_insts


def get_nc(cfg=DTYPE_CFG):
    if cfg not in _CACHE:
        _CACHE[cfg] = _build_nc(cfg)
    return _CACHE[cfg]


# ---------- host-side packing ----------

def _pack_masks(m, np_dt):
    # m: (BL, K, H, W) float32 -> (BL, G, MR=8q+k, FREE=it*N+n)
    a = m.reshape(BL, K, G, SUB, C, N).astype(np_dt)        # b k g it q n
    a = a.transpose(0, 2, 4, 1, 3, 5)                       # b g q k it n
    return np.ascontiguousarray(a.reshape(BL, G, MR, FREE))


def _pack_points(p, np_dt):
    # p: (BL, 3, H, W) -> (BL, G, PR, FREE); rows 9q+3j+r = p_j (r replicated), rows 90+ = 1.0
    a = p.reshape(BL, 3, G, SUB, C, N).astype(np_dt)        # b j g it q n
    a = a.transpose(0, 2, 4, 1, 3, 5)                       # b g q j it n
    a = np.broadcast_to(a[:, :, :, :, None, :, :], (BL, G, C, 3, 3, SUB, N))  # b g q j r it n
    a = a.reshape(BL, G, RP, FREE)
    ones = np.ones((BL, G, PR - RP, FREE), dtype=np_dt)
    return np.ascontiguousarray(np.concatenate([a, ones], axis=2))


def _make_cbd(t, np_dt):
    # t: (BL, K, 3, 4) transforms -> (BL, MR, PR)
    cbd = np.zeros((BL, MR, PR), dtype=np.float32)
    rot = t[:, :, :, :3].transpose(0, 1, 3, 2).reshape(BL, K, 9)   # b k (j r): value T[b,k,r,j]
    tr = t[:, :, :, 3]                                             # b k i
    for q in range(C):
        cbd[:, 8 * q:8 * q + 8, 9 * q:9 * q + 9] = rot
        cbd[:, 8 * q:8 * q + 8, RP + 3 * q:RP + 3 * q + 3] = tr
    return cbd.astype(np_dt)


def _make_obd(np_dt):
    obd = np.zeros((PR, OR), dtype=np.float32)
    for q in range(C):
        for i in range(3):
            for j in range(3):
                obd[9 * q + 3 * j + i, 3 * q + i] = 1.0
            obd[RP + 3 * q + i, 3 * q + i] = 1.0
    return obd.astype(np_dt)


def _unpack_out(o):
    # o: (BL, G, OR=3q+i, FREE=it*N+n) -> (BL, 3, H, W) float32
    a = np.asarray(o).reshape(BL, G, C, 3, SUB, N)          # b g q i it n
    a = a.transpose(0, 3, 1, 4, 2, 5)                       # b i g it q n
    return a.reshape(BL, 3, H, W).astype(np.float32)


def make_in_maps(points, masks, transforms, cfg=DTYPE_CFG):
    np_dt, _ = _dts(cfg)
    points = np.asarray(points, dtype=np.float32)
    masks = np.asarray(masks, dtype=np.float32)
    transforms = np.asarray(transforms, dtype=np.float32)
    obd = _make_obd(np_dt)
    in_maps = []
    for core in range(NCORES):
        b0 = core * BL
        in_maps.append({
            "masks_p": _pack_masks(masks[b0:b0 + BL], np_dt),
            "pts_p": _pack_points(points[b0:b0 + BL], np_dt),
            "cbd": _make_cbd(transforms[b0:b0 + BL], np_dt),
            "obd": obd,
        })
    return in_maps


def kernel(points, masks, transforms, cfg=DTYPE_CFG, **run_kwargs):
    nc = get_nc(cfg)
    in_maps = make_in_maps(points, masks, transforms, cfg)
    res = run_bass_kernel_spmd(nc, in_maps, list(range(NCORES)), **run_kwargs)
    outs = [_unpack_out(res.results[c]["out_p"]) for c in range(NCORES)]
    full = np.concatenate(outs, axis=0)
    kernel._last_results = res
    return full


# revision 17
# speedup vs baseline: 2.0855x; 1.0045x over previous
"""Trainium2 Bass kernel for nn_NTfm3D: out[b,i,h,w] = sum_k masks[b,k,h,w] * (R[b,k] @ p[b,:,h,w] + t[b,k]).

Strategy (per core, data-parallel over batch B=16 across 8 cores, B_local=2):

Algebra:  out_i = sum_j S_ij * p~_j   where  S_ij = sum_k T[b,k,i,j] * m_k  (p~ = (x,y,z,1))
  mm1 (TensorE):  S = C_bd^T @ masks      (contract 8 masks per chunk, block-diag over 10 chunks)
  TT  (VectorE):  F = S * P3              (per-pixel multiply; P3 = points replicated x3 + ones rows)
  mm2 (TensorE):  out = O_bd^T @ F        (contract j, block-diag 0/1 weights)
  ACT (ScalarE):  psum -> sbuf staging;   DMA out.

Layout: channel-major tiles, N=512 pixel columns per matmul, C=10 chunks stacked on
partitions (S/P3/F use 120 partitions, masks 80). Host pre-packs all tiles into
contiguous per-group blocks (G groups of SUB sub-iters) so DMAs are large & contiguous.
"""

import numpy as np
from contextlib import ExitStack

try:  # persistent jax compile cache: avoids recompiling the NEFF across processes
    import jax as _jax
    _jax.config.update("jax_compilation_cache_dir", "/tmp/jaxcache")
except Exception:
    pass

import concourse.bass as bass
import concourse.tile as tile
from concourse import mybir
from concourse.bass_utils import run_bass_kernel_spmd

# ---- problem constants (hardcoded per contract) ----
B, K, H, W = 16, 8, 480, 640
HW = H * W                      # 307200
NCORES = 8
BL = B // NCORES                # 2 batches per core
N = 512                         # pixel columns per matmul
C = 10                          # chunks stacked per sub-iteration
SUB = 15                        # sub-iterations per group (one DMA batch)
G = HW // (N * C * SUB)         # 4 groups per batch plane
assert G * SUB * C * N == HW
FREE = SUB * N                  # 7680 free columns per group tile
RP = 9 * C                      # 90 product rows
PR = 12 * C                     # 120 partitions for S/P3/F
MR = 8 * C                      # 80 mask rows
OR = 3 * C                      # 30 output rows

# dtype config: "f16" (fast path) or "f32" (exact path)
DTYPE_CFG = "f16"

_CACHE = {}


def _dts(cfg):
    if cfg == "f16":
        return np.float16, mybir.dt.float16
    return np.float32, mybir.dt.float32


def _build_nc(cfg, loop_repeat=None):
    np_dt, my_dt = _dts(cfg)
    f32 = mybir.dt.float32
    out_my = my_dt  # staging/output dtype matches input dtype config

    nc = bass.Bass()
    masks_d = nc.declare_dram_parameter("masks_p", [BL, G, MR, FREE], my_dt, isOutput=False)
    pts_d = nc.declare_dram_parameter("pts_p", [BL, G, PR, FREE], my_dt, isOutput=False)
    cbd_d = nc.declare_dram_parameter("cbd", [BL, MR, PR], my_dt, isOutput=False)
    obd_d = nc.declare_dram_parameter("obd", [PR, OR], my_dt, isOutput=False)
    out_d = nc.declare_dram_parameter("out_p", [BL, G, OR, FREE], out_my, isOutput=True)

    with ExitStack() as ctx:
        tc = ctx.enter_context(tile.TileContext(nc))
        wpool = ctx.enter_context(tc.tile_pool(name="wpool", bufs=1))
        cpool = ctx.enter_context(tc.tile_pool(name="cpool", bufs=2))
        mpool = ctx.enter_context(tc.tile_pool(name="mpool", bufs=2))
        ppool = ctx.enter_context(tc.tile_pool(name="ppool", bufs=2))
        fpool = ctx.enter_context(tc.tile_pool(name="fpool", bufs=4))
        ospool = ctx.enter_context(tc.tile_pool(name="ospool", bufs=2))
        s_psum = ctx.enter_context(tc.tile_pool(name="s_psum", bufs=3, space=bass.MemorySpace.PSUM))
        o_psum = ctx.enter_context(tc.tile_pool(name="o_psum", bufs=3, space=bass.MemorySpace.PSUM))

        obd_t = wpool.tile([PR, OR], my_dt)
        nc.gpsimd.dma_start(out=obd_t[:], in_=obd_d[:])

        def body():
            for b in range(BL):
                cbd_t = cpool.tile([MR, PR], my_dt)
                nc.gpsimd.dma_start(out=cbd_t[:], in_=cbd_d[b])
                for g in range(G):
                    mk = mpool.tile([MR, FREE], my_dt)
                    nc.gpsimd.dma_start(out=mk[:], in_=masks_d[b, g])
                    pt = ppool.tile([PR, FREE], my_dt)
                    nc.gpsimd.dma_start(out=pt[:], in_=pts_d[b, g])
                    ost = ospool.tile([OR, FREE], out_my)
                    for it in range(SUB):
                        sl = slice(it * N, (it + 1) * N)
                        s_ps = s_psum.tile([PR, N], f32)
                        nc.tensor.matmul(s_ps[:], cbd_t[:], mk[:, sl], start=True, stop=True)
                        fp = fpool.tile([PR, N], my_dt)
                        nc.vector.tensor_mul(fp[:], s_ps[:], pt[:, sl])
                        o_ps = o_psum.tile([OR, N], f32)
                        nc.tensor.matmul(o_ps[:], obd_t[:], fp[:], start=True, stop=True)
                        nc.scalar.copy(ost[:, sl], o_ps[:])
                    nc.gpsimd.dma_start(out=out_d[b, g], in_=ost[:])

        if loop_repeat is None:
            body()
        else:
            with tc.For_i(0, loop_repeat, 1):
                body()
    _split_excess_waits(nc)
    return nc


# Walrus's per-instruction sync-wait slots are limited (DVE tensor ops fit only ONE).
# Engine queues are strict FIFO, so hoisting extra waits onto injected same-engine
# NoOps immediately before the instruction is semantically identical.
_WAIT_LIMIT = {}
_WAIT_LIMIT_DEFAULT = 1
_SKIP_TYPES = {"InstEventSemaphore", "InstUnconditionalBranch"}


def _split_excess_waits(nc):
    uid = [0]
    for fn in nc.m.functions:
        for blk in fn.blocks:
            new_insts = []
            for inst in blk.instructions:
                nm = type(inst).__name__
                si = getattr(inst, "sync_info", None)
                eng = getattr(inst, "engine", None)
                if (
                    si is not None
                    and si.on_wait
                    and nm not in _SKIP_TYPES
                    and eng is not None
                ):
                    limit = _WAIT_LIMIT.get(nm, _WAIT_LIMIT_DEFAULT)
                    waits = list(si.on_wait)
                    if len(waits) > limit:
                        keep = waits[-limit:]
                        extra = waits[:-limit]
                        for w in extra:
                            uid[0] += 1
                            nop = mybir.InstNoOp(
                                name=f"wsplit-{uid[0]}",
                                engine=eng,
                                sync_info=mybir.SyncInfo(on_wait=[w], on_update=[]),
                                bass_nofuse=True,
                            )
                            new_insts.append(nop)
                        inst.sync_info = mybir.SyncInfo(on_wait=keep, on_update=list(si.on_update))
                new_insts.append(inst)
            blk.instructions = new_insts


def get_nc(cfg=DTYPE_CFG):
    if cfg not in _CACHE:
        _CACHE[cfg] = _build_nc(cfg)
    return _CACHE[cfg]


# ---------- host-side packing ----------

def _pack_masks(m, np_dt):
    # m: (BL, K, H, W) float32 -> (BL, G, MR=8q+k, FREE=it*N+n)
    a = m.reshape(BL, K, G, SUB, C, N).astype(np_dt)        # b k g it q n
    a = a.transpose(0, 2, 4, 1, 3, 5)                       # b g q k it n
    return np.ascontiguousarray(a.reshape(BL, G, MR, FREE))


def _pack_points(p, np_dt):
    # p: (BL, 3, H, W) -> (BL, G, PR, FREE); rows 9q+3j+r = p_j (r replicated), rows 90+ = 1.0
    a = p.reshape(BL, 3, G, SUB, C, N).astype(np_dt)        # b j g it q n
    a = a.transpose(0, 2, 4, 1, 3, 5)                       # b g q j it n
    a = np.broadcast_to(a[:, :, :, :, None, :, :], (BL, G, C, 3, 3, SUB, N))  # b g q j r it n
    a = a.reshape(BL, G, RP, FREE)
    ones = np.ones((BL, G, PR - RP, FREE), dtype=np_dt)
    return np.ascontiguousarray(np.concatenate([a, ones], axis=2))


def _make_cbd(t, np_dt):
    # t: (BL, K, 3, 4) transforms -> (BL, MR, PR)
    cbd = np.zeros((BL, MR, PR), dtype=np.float32)
    rot = t[:, :, :, :3].transpose(0, 1, 3, 2).reshape(BL, K, 9)   # b k (j r): value T[b,k,r,j]
    tr = t[:, :, :, 3]                                             # b k i
    for q in range(C):
        cbd[:, 8 * q:8 * q + 8, 9 * q:9 * q + 9] = rot
        cbd[:, 8 * q:8 * q + 8, RP + 3 * q:RP + 3 * q + 3] = tr
    return cbd.astype(np_dt)


def _make_obd(np_dt):
    obd = np.zeros((PR, OR), dtype=np.float32)
    for q in range(C):
        for i in range(3):
            for j in range(3):
                obd[9 * q + 3 * j + i, 3 * q + i] = 1.0
            obd[RP + 3 * q + i, 3 * q + i] = 1.0
    return obd.astype(np_dt)


def _unpack_out(o):
    # o: (BL, G, OR=3q+i, FREE=it*N+n) -> (BL, 3, H, W) float32
    a = np.asarray(o).reshape(BL, G, C, 3, SUB, N)          # b g q i it n
    a = a.transpose(0, 3, 1, 4, 2, 5)                       # b i g it q n
    return a.reshape(BL, 3, H, W).astype(np.float32)


def make_in_maps(points, masks, transforms, cfg=DTYPE_CFG):
    np_dt, _ = _dts(cfg)
    points = np.asarray(points, dtype=np.float32)
    masks = np.asarray(masks, dtype=np.float32)
    transforms = np.asarray(transforms, dtype=np.float32)
    obd = _make_obd(np_dt)
    in_maps = []
    for core in range(NCORES):
        b0 = core * BL
        in_maps.append({
            "masks_p": _pack_masks(masks[b0:b0 + BL], np_dt),
            "pts_p": _pack_points(points[b0:b0 + BL], np_dt),
            "cbd": _make_cbd(transforms[b0:b0 + BL], np_dt),
            "obd": obd,
        })
    return in_maps


def kernel(points, masks, transforms, cfg=DTYPE_CFG, **run_kwargs):
    nc = get_nc(cfg)
    in_maps = make_in_maps(points, masks, transforms, cfg)
    res = run_bass_kernel_spmd(nc, in_maps, list(range(NCORES)), **run_kwargs)
    outs = [_unpack_out(res.results[c]["out_p"]) for c in range(NCORES)]
    full = np.concatenate(outs, axis=0)
    kernel._last_results = res
    return full
